# revision 8
# baseline (speedup 1.0000x reference)
"""Trainium2 Bass kernel: dense transformer attention block (QKV proj + RoPE +
GQA causal attention + output proj), tensor-parallel over 8 NeuronCores.

Sharding: heads are split across cores (4 Q heads + 1 KV head per core).
Single fully-interleaved pipeline: QKV-projection token chunks, attention
q-chunks (forward order, so chunk qc only needs K/V through token
256*(qc+1)), per-chunk AllGathers of the head-sharded attention output, and
o_proj column-slice chunks are all emitted interleaved so the Tile
scheduler can pack the TensorEngine with projection matmuls inside the
softmax (scalar-EXP) latency gaps and spread the AllGather stream across
the whole kernel instead of serializing it at the end.  K and V^T live in
SBUF (V transposed on the PE via identity-matmul); only Q bounces through
HBM.  Softmax column-sum accumulation runs on the (otherwise idle) GpSimd
engine.
"""

from contextlib import ExitStack

import numpy as np
import ml_dtypes

import concourse.bass as bass
from concourse import bacc
import concourse.tile as tile
import concourse.mybir as mybir
from concourse.bass_utils import run_bass_kernel_spmd

F32 = mybir.dt.float32
F32R = mybir.dt.float32r
BF16 = mybir.dt.bfloat16
EXP = mybir.ActivationFunctionType.Exp
SCALE = float(128 ** -0.5)

N_CORES = 8
N_HEADS = 32
N_KV_HEADS = 8
D = 128          # head dim
HID = 4096
B = 2
S = 2048
T = B * S        # 4096 tokens
ROPE_BASE = 10000.0

HL = N_HEADS // N_CORES          # 4 local Q heads per core
QKV_ROWS = (HL + 2) * D          # 768: 4 Q heads + 1 K head + 1 V head
JC = HID // N_CORES              # 512 output columns per core

TC = 256                         # token chunk for the QKV projection phase
QC = 256                         # query chunk in attention
N_HT = HID // 128                # 32 hidden tiles
N_QC = S // QC                   # 8 q-chunks per batch
AGC = 256                        # AllGather chunk granularity (tokens)
N_CH = T // AGC                  # 16 AllGather chunks
OPROJ_LAG = 3                    # o_proj trails the AG stream by this many


def _emit(tc_ctx, xt, wqkvt, wot, ropes, out_t, qkt, ag_ins, ag_outs):
    nc = tc_ctx.nc
    n_tc = T // TC           # 16 projection chunks
    n_kt = S // 128          # 16 k-tiles per batch

    with ExitStack() as es:
        const_pool = es.enter_context(tc_ctx.tile_pool(name="const", bufs=1))
        # All-ones stationary: one matmul computes column sums AND broadcasts
        # them across all 128 partitions.
        ones_mat = const_pool.tile([128, 128], F32R)
        # Identity (bf16) for PE-transposing V tiles.
        ident = const_pool.tile([128, 128], BF16)
        # Diagonal causal masks: mask_d[k, q] = 1.0 if q - k - 128*d >= 0.
        masks = const_pool.tile([128, 2, QC], BF16)
        nc.vector.memset(masks, 1.0)
        nc.vector.tensor_copy(ones_mat, masks[:, 0, 0:128])
        nc.vector.tensor_copy(ident, masks[:, 0, 0:128])
        nc.gpsimd.affine_select(
            out=ident, in_=ident,
            compare_op=mybir.AluOpType.is_equal,
            fill=0.0, base=0, pattern=[[1, 128]], channel_multiplier=-1,
        )
        for d_off in range(2):
            nc.gpsimd.affine_select(
                out=masks[:, d_off, :],
                in_=masks[:, d_off, :],
                compare_op=mybir.AluOpType.is_ge,
                fill=0.0,
                base=-128 * d_off,
                pattern=[[1, QC]],
                channel_multiplier=-1,
            )

        # ------------------------- persistent pools -------------------------
        wpool = es.enter_context(tc_ctx.tile_pool(name="wq", bufs=1))
        wopool = es.enter_context(tc_ctx.tile_pool(name="wo", bufs=1))
        xpool = es.enter_context(tc_ctx.tile_pool(name="x", bufs=2))
        rpool = es.enter_context(tc_ctx.tile_pool(name="rope", bufs=2))
        shpool = es.enter_context(tc_ctx.tile_pool(name="rsh", bufs=2))
        opool = es.enter_context(tc_ctx.tile_pool(name="p1out", bufs=2))
        kpool = es.enter_context(tc_ctx.tile_pool(name="ksb", bufs=2))
        vpool = es.enter_context(tc_ctx.tile_pool(name="vsb", bufs=2))
        vdtpool = es.enter_context(tc_ctx.tile_pool(name="vdt", bufs=2))
        qpool = es.enter_context(tc_ctx.tile_pool(name="qld", bufs=2))
        ptpool = es.enter_context(tc_ctx.tile_pool(name="pt", bufs=4))
        cspool = es.enter_context(tc_ctx.tile_pool(name="cs", bufs=2))
        mpool = es.enter_context(tc_ctx.tile_pool(name="misc", bufs=2))
        apool = es.enter_context(tc_ctx.tile_pool(name="ag", bufs=2))
        respool = es.enter_context(tc_ctx.tile_pool(name="res", bufs=2))
        # PSUM: 8 banks total.  acc (3, shared proj/o_proj) + scores (2,
        # also reused for the softmax-sum broadcast) + V-transpose (1) +
        # AV accumulator (2).
        ps_acc = es.enter_context(
            tc_ctx.tile_pool(name="ps_acc", bufs=3, space="PSUM"))
        ps_s = es.enter_context(
            tc_ctx.tile_pool(name="ps_s", bufs=2, space="PSUM"))
        ps_o = es.enter_context(
            tc_ctx.tile_pool(name="ps_o", bufs=2, space="PSUM"))
        ps_t = es.enter_context(
            tc_ctx.tile_pool(name="ps_t", bufs=1, space="PSUM"))

        wq_sb = wpool.tile([128, HL + 2, N_HT, 128], BF16)
        for ot in range(HL + 2):
            # Weights ride the scalar-engine HWDGE queue so the first X
            # chunk (sync queue) lands in parallel.
            nc.scalar.dma_start(out=wq_sb[:, ot], in_=wqkvt.ap()[:, ot])
        wo_sb = wopool.tile([128, N_HT, JC], BF16)
        nc.scalar.dma_start(out=wo_sb, in_=wot.ap())

        k_sb = {}                # per-batch [128, S] bf16 (d on partitions)
        v_sb = {}                # per-batch [128, n_kt, 128] bf16 ([t, d])

        # ---------------- QKV projection + RoPE (one chunk) -----------------
        def proj_chunk(ch):
            b = ch // (S // TC)
            cb = ch % (S // TC)          # chunk index within batch
            if cb == 0:
                k_sb[b] = kpool.tile([128, S], BF16, name=f"k_sb{b}")
                v_sb[b] = vpool.tile([128, n_kt, 128], BF16, name=f"v_sb{b}")
            xt_sb = xpool.tile([128, N_HT, TC], BF16)
            for hq in range(2):
                nc.sync.dma_start(
                    out=xt_sb[:, hq * 16:(hq + 1) * 16, :],
                    in_=xt.ap()[:, ch, hq * 16:(hq + 1) * 16, :],
                )
            rope_sb = rpool.tile([128, 2, TC], F32)
            nc.sync.dma_start(out=rope_sb, in_=ropes.ap()[:, ch])
            for ot in range(HL + 2):
                ps = ps_acc.tile([128, TC], F32, tag="acc")
                for h in range(N_HT):
                    nc.tensor.matmul(
                        ps,
                        lhsT=wq_sb[:, ot, h, :],
                        rhs=xt_sb[:, h, :],
                        start=(h == 0),
                        stop=(h == N_HT - 1),
                    )
                if ot < HL:
                    # RoPE for Q; bounce to HBM (qkt), reloaded per q-chunk.
                    sh = shpool.tile([128, TC], F32, tag="sh")
                    nc.vector.tensor_copy(sh[0:64, :], ps[64:128, :])
                    nc.vector.tensor_copy(sh[64:128, :], ps[0:64, :])
                    nc.vector.tensor_mul(sh, sh, rope_sb[:, 1, :])
                    tmp = shpool.tile([128, TC], F32, tag="tmp")
                    nc.vector.tensor_mul(tmp, ps, rope_sb[:, 0, :])
                    qk_out = opool.tile([128, TC], BF16, tag="qk")
                    nc.vector.tensor_add(qk_out, tmp, sh)
                    nc.sync.dma_start(
                        out=qkt[ot * 128:(ot + 1) * 128,
                                ch * TC:(ch + 1) * TC],
                        in_=qk_out,
                    )
                elif ot == HL:
                    # RoPE for K; written straight into the SBUF-resident
                    # per-batch K tile ([d, t] layout).
                    sh = shpool.tile([128, TC], F32, tag="sh")
                    nc.vector.tensor_copy(sh[0:64, :], ps[64:128, :])
                    nc.vector.tensor_copy(sh[64:128, :], ps[0:64, :])
                    nc.vector.tensor_mul(sh, sh, rope_sb[:, 1, :])
                    tmp = shpool.tile([128, TC], F32, tag="tmp")
                    nc.vector.tensor_mul(tmp, ps, rope_sb[:, 0, :])
                    nc.vector.tensor_add(
                        k_sb[b][:, cb * TC:(cb + 1) * TC], tmp, sh)
                else:
                    # V head: [d, t] -> PE-transpose 128x128 tiles -> [t, d].
                    v_dt = vdtpool.tile([128, TC], BF16)
                    nc.vector.tensor_copy(v_dt, ps)
                    pst = ps_t.tile([128, 2, 128], BF16, tag="pst")
                    for i in range(2):
                        nc.tensor.transpose(
                            pst[:, i, :], v_dt[:, i * 128:(i + 1) * 128],
                            ident)
                        nc.vector.tensor_copy(
                            v_sb[b][:, cb * 2 + i, :], pst[:, i, :])

        # ------------- attention q-chunk (4 local heads) + AG ---------------
        pace = {"inst": None}
        ags_fired = []
        oproj_done = []

        def attn_group(b, qc):
            chunk = b * N_QC + qc
            kt_max = 2 * (qc + 1)
            q_sbt = qpool.tile([128, HL, QC], BF16)
            nc.sync.dma_start(
                out=q_sbt,
                in_=qkt.rearrange("(hh p) t -> p hh t", p=128)[
                    :, 0:HL, b * S + qc * QC:b * S + (qc + 1) * QC],
            )
            for hh in range(HL):
                pso = ps_o.tile([128, QC], F32, tag="pso")
                colsum = cspool.tile([128, QC], F32R, tag="cs")
                for kt in range(kt_max):
                    d_off = kt - 2 * qc
                    ps = ps_s.tile([128, QC], F32, tag="s")
                    nc.tensor.matmul(
                        ps,
                        lhsT=k_sb[b][:, kt * 128:(kt + 1) * 128],
                        rhs=q_sbt[:, hh, :],
                        start=True,
                        stop=True,
                    )
                    pt = ptpool.tile([128, QC], BF16)
                    nc.scalar.activation(pt, ps, EXP, scale=SCALE)
                    if d_off >= 0:
                        nc.vector.tensor_mul(pt, pt, masks[:, d_off, :])
                    nc.tensor.matmul(
                        pso,
                        lhsT=v_sb[b][:, kt, :],
                        rhs=pt,
                        start=(kt == 0),
                        stop=(kt == kt_max - 1),
                    )
                    # Column-sum accumulation on the (otherwise idle) GpSimd.
                    if kt == 0:
                        nc.gpsimd.tensor_copy(colsum, pt)
                    else:
                        nc.gpsimd.tensor_add(colsum, colsum, pt)
                # One matmul against an all-ones stationary both sums over k
                # and broadcasts the sums to all partitions (reuses a scores
                # slot).
                sums_bc = ps_s.tile([128, QC], F32, tag="s", name="sums_bc")
                nc.tensor.matmul(
                    sums_bc,
                    lhsT=ones_mat,
                    rhs=colsum,
                    start=True,
                    stop=True,
                )
                recip_bc = mpool.tile([128, QC], F32, tag="recip_bc")
                rscr = mpool.tile([128, QC], F32, tag="rscr")
                nc.vector.reciprocal_approx_accurate(recip_bc, sums_bc, rscr)
                attn = mpool.tile([128, QC], BF16, tag="attn")
                nc.vector.tensor_mul(attn, pso, recip_bc)
                wr = nc.sync.dma_start(
                    out=ag_ins[chunk][hh * 128:(hh + 1) * 128, :],
                    in_=attn,
                )
                pace["inst"] = wr.ins
            nc.gpsimd.collective_compute(
                "AllGather",
                mybir.AluOpType.bypass,
                replica_groups=[list(range(N_CORES))],
                ins=[ag_ins[chunk][:]],
                outs=[ag_outs[chunk][:]],
            )
            ags_fired.append(chunk)

        # --------------------- o_proj (one AG chunk) ------------------------
        def emit_oproj(k):
            t0 = k * AGC
            ag_sbt = apool.tile([128, N_HT, AGC], BF16)
            ld = nc.sync.dma_start(
                out=ag_sbt,
                in_=ag_outs[k].rearrange("(ht p) t -> p ht t", p=128),
            )
            if pace["inst"] is not None:
                # Pace AllGather-output consumption behind real attention
                # progress; the scheduler's collective latency estimate is
                # optimistic and otherwise stalls the in-order PE stream.
                bass._add_dep_helper(
                    ld.ins, pace["inst"], sync=True,
                    reason="oproj paced behind attention",
                )
            for jt in range(JC // 128):
                ps4 = ps_acc.tile([128, AGC], F32, tag="acc", name="ps4")
                for h in range(N_HT):
                    nc.tensor.matmul(
                        ps4,
                        lhsT=wo_sb[:, h, jt * 128:(jt + 1) * 128],
                        rhs=ag_sbt[:, h, :],
                        start=(h == 0),
                        stop=(h == N_HT - 1),
                    )
                res4 = respool.tile([128, AGC], BF16)
                nc.vector.tensor_copy(res4, ps4)
                nc.sync.dma_start(
                    out=out_t[jt * 128:(jt + 1) * 128, t0:t0 + AGC],
                    in_=res4,
                )
            oproj_done.append(k)

        # ------------------------- emission schedule ------------------------
        # P(ch): projection chunk; A(b,qc): attention q-chunk (needs proj
        # chunks 8b..8b+qc for K/V and 8b+qc for Q -> always >=2 chunks of
        # slack here).  o_proj k trails the AG stream by OPROJ_LAG.
        plan = []
        for ch in range(4):
            plan.append(("P", ch))
        next_ch = 4
        for b in range(B):
            for qc in range(N_QC):
                plan.append(("A", b, qc))
                if next_ch < n_tc:
                    plan.append(("P", next_ch))
                    next_ch += 1
        # Reorder: keep P-chunks flowing a bit ahead of the A-groups they
        # feed.  plan above gives A(b,qc) after P(8b+qc+3).
        for item in plan:
            if item[0] == "P":
                proj_chunk(item[1])
            else:
                attn_group(item[1], item[2])
                if len(ags_fired) > OPROJ_LAG:
                    emit_oproj(ags_fired[-1 - OPROJ_LAG])
        for k in range(N_CH):
            if k not in oproj_done:
                emit_oproj(k)


def _build_program():
    nc = bacc.Bacc("TRN2", target_bir_lowering=False, debug=False,
                   num_devices=N_CORES)
    xt = nc.declare_dram_parameter("xt", [128, T // TC, N_HT, TC], BF16,
                                   isOutput=False)
    wqkvt = nc.declare_dram_parameter("wqkvt", [128, HL + 2, N_HT, 128], BF16,
                                      isOutput=False)
    wot = nc.declare_dram_parameter("wot", [128, N_HT, JC], BF16,
                                    isOutput=False)
    ropes = nc.declare_dram_parameter("ropes", [128, T // TC, 2, TC], F32,
                                      isOutput=False)
    out_t = nc.declare_dram_parameter("out_t", [JC, T], BF16, isOutput=True)

    qkt = nc.dram_tensor("qkt", [HL * D, T], BF16).ap()
    ag_ins = [nc.dram_tensor(f"ag_in{k}", [HL * D, AGC], BF16).ap()
              for k in range(N_CH)]
    ag_outs = [nc.dram_tensor(f"ag_out{k}", [N_HEADS * D, AGC], BF16,
                              addr_space="Shared").ap()
               for k in range(N_CH)]

    with tile.TileContext(nc) as tc_ctx:
        _emit(tc_ctx, xt, wqkvt, wot, ropes, out_t, qkt, ag_ins, ag_outs)
    nc.finalize()
    return nc


def _host_inputs(hidden_states, w_qkv, w_o):
    """Shard + transpose inputs for the 8 cores; returns in_maps."""
    X = np.asarray(hidden_states, dtype=np.float32).reshape(T, HID)
    # [p, ch, ht, tc] tiled layout so every DMA line is contiguous.
    xt = np.ascontiguousarray(
        X.reshape(T // TC, TC, N_HT, 128).transpose(3, 0, 2, 1)
    ).astype(ml_dtypes.bfloat16)

    # RoPE tables in [d, t] layout with rotate-half sign folded into sin and
    # the attention scale folded into the Q tables.
    inv_freq = 1.0 / (ROPE_BASE ** (np.arange(0, D, 2, dtype=np.float32) / D))
    pos = np.arange(S, dtype=np.float32)
    freqs = np.outer(pos, inv_freq)                      # (S, D/2)
    emb = np.concatenate([freqs, freqs], axis=-1)        # (S, D)
    cos = np.cos(emb).T.astype(np.float32)               # (D, S)
    sin = np.sin(emb).T.astype(np.float32)
    sgn = np.concatenate([-np.ones(D // 2), np.ones(D // 2)]).astype(np.float32)
    sins = sgn[:, None] * sin
    cos_t = np.tile(cos, (1, B))                         # (D, T)
    sins_t = np.tile(sins, (1, B))
    ropes = np.stack([cos_t, sins_t], axis=0)
    ropes = np.ascontiguousarray(
        ropes.reshape(2, 128, T // TC, TC).transpose(1, 2, 0, 3),
        dtype=np.float32,
    )

    w_qkv = np.asarray(w_qkv, dtype=np.float32)
    w_o = np.asarray(w_o, dtype=np.float32)
    q_sz = N_HEADS * D
    kv_sz = N_KV_HEADS * D
    in_maps = []
    for c in range(N_CORES):
        qr = w_qkv[c * HL * D:(c + 1) * HL * D]
        kr = w_qkv[q_sz + c * D:q_sz + (c + 1) * D]
        vr = w_qkv[q_sz + kv_sz + c * D:q_sz + kv_sz + (c + 1) * D]
        w_shard = np.concatenate([qr, kr, vr], axis=0)           # (768, HID)
        wqkvt_c = np.ascontiguousarray(
            w_shard.reshape(HL + 2, 128, N_HT, 128).transpose(3, 0, 2, 1)
        ).astype(ml_dtypes.bfloat16)
        wot_c = np.ascontiguousarray(
            w_o[c * JC:(c + 1) * JC, :].reshape(JC, N_HT, 128).transpose(2, 1, 0)
        ).astype(ml_dtypes.bfloat16)
        in_maps.append({
            "xt": xt, "wqkvt": wqkvt_c, "wot": wot_c, "ropes": ropes,
        })
    return in_maps


def _run(hidden_states, w_qkv, w_o, trace=False, tmpdir=None):
    in_maps = _host_inputs(hidden_states, w_qkv, w_o)
    nc = _build_program()
    res = run_bass_kernel_spmd(nc, in_maps, list(range(N_CORES)),
                               trace=trace, tmpdir=tmpdir)
    out_T = np.concatenate(
        [np.asarray(res.results[c]["out_t"]).astype(np.float32)
         for c in range(N_CORES)], axis=0
    )                                                     # (HID j, T)
    out = np.ascontiguousarray(out_T.T).reshape(B, S, HID).astype(np.float32)
    return out, res


def kernel(hidden_states, w_qkv, w_o):
    out, _ = _run(hidden_states, w_qkv, w_o, trace=False)
    return out


# revision 10
# speedup vs baseline: 1.0383x; 1.0383x over previous
"""Trainium2 Bass kernel: dense transformer attention block (QKV proj + RoPE +
GQA causal attention + output proj), tensor-parallel over 8 NeuronCores.

Sharding: heads are split across cores (4 Q heads + 1 KV head per core).
Single fully-interleaved pipeline: QKV-projection token chunks, attention
q-chunks (forward order, so chunk qc only needs K/V through token
256*(qc+1)), per-chunk AllGathers of the head-sharded attention output, and
o_proj column-slice chunks are all emitted interleaved so the Tile
scheduler can pack the TensorEngine with projection matmuls inside the
softmax (scalar-EXP) latency gaps and spread the AllGather stream across
the whole kernel instead of serializing it at the end.  K and V^T live in
SBUF (V transposed on the PE via identity-matmul); only Q bounces through
HBM.  Softmax column-sum accumulation runs on the (otherwise idle) GpSimd
engine.
"""

from contextlib import ExitStack

import numpy as np
import ml_dtypes

import concourse.bass as bass
from concourse import bacc
import concourse.tile as tile
import concourse.mybir as mybir
from concourse.bass_utils import run_bass_kernel_spmd

F32 = mybir.dt.float32
F32R = mybir.dt.float32r
BF16 = mybir.dt.bfloat16
EXP = mybir.ActivationFunctionType.Exp
SCALE = float(128 ** -0.5)

N_CORES = 8
N_HEADS = 32
N_KV_HEADS = 8
D = 128          # head dim
HID = 4096
B = 2
S = 2048
T = B * S        # 4096 tokens
ROPE_BASE = 10000.0

HL = N_HEADS // N_CORES          # 4 local Q heads per core
QKV_ROWS = (HL + 2) * D          # 768: 4 Q heads + 1 K head + 1 V head
JC = HID // N_CORES              # 512 output columns per core

TC = 256                         # token chunk for the QKV projection phase
QC = 256                         # query chunk in attention
N_HT = HID // 128                # 32 hidden tiles
N_QC = S // QC                   # 8 q-chunks per batch
AGC = 256                        # AllGather chunk granularity (tokens)
N_CH = T // AGC                  # 16 AllGather chunks
OPROJ_LAG = 3                    # o_proj trails the AG stream by this many


def _emit(tc_ctx, xt, wqkvt, wot, ropes, out_t, qkt, ag_ins, ag_outs):
    nc = tc_ctx.nc
    n_tc = T // TC           # 16 projection chunks
    n_kt = S // 128          # 16 k-tiles per batch

    with ExitStack() as es:
        const_pool = es.enter_context(tc_ctx.tile_pool(name="const", bufs=1))
        # All-ones stationary: one matmul computes column sums AND broadcasts
        # them across all 128 partitions.
        ones_mat = const_pool.tile([128, 128], F32R)
        # Identity (bf16) for PE-transposing V tiles.
        ident = const_pool.tile([128, 128], BF16)
        # Diagonal causal masks: mask_d[k, q] = 1.0 if q - k - 128*d >= 0.
        masks = const_pool.tile([128, 2, QC], BF16)
        nc.vector.memset(masks, 1.0)
        nc.vector.tensor_copy(ones_mat, masks[:, 0, 0:128])
        nc.vector.tensor_copy(ident, masks[:, 0, 0:128])
        nc.gpsimd.affine_select(
            out=ident, in_=ident,
            compare_op=mybir.AluOpType.is_equal,
            fill=0.0, base=0, pattern=[[1, 128]], channel_multiplier=-1,
        )
        for d_off in range(2):
            nc.gpsimd.affine_select(
                out=masks[:, d_off, :],
                in_=masks[:, d_off, :],
                compare_op=mybir.AluOpType.is_ge,
                fill=0.0,
                base=-128 * d_off,
                pattern=[[1, QC]],
                channel_multiplier=-1,
            )

        # ------------------------- persistent pools -------------------------
        wpool = es.enter_context(tc_ctx.tile_pool(name="wq", bufs=1))
        wopool = es.enter_context(tc_ctx.tile_pool(name="wo", bufs=1))
        xpool = es.enter_context(tc_ctx.tile_pool(name="x", bufs=2))
        rpool = es.enter_context(tc_ctx.tile_pool(name="rope", bufs=1))
        shpool = es.enter_context(tc_ctx.tile_pool(name="rsh", bufs=2))
        opool = es.enter_context(tc_ctx.tile_pool(name="p1out", bufs=2))
        kpool = es.enter_context(tc_ctx.tile_pool(name="ksb", bufs=2))
        vpool = es.enter_context(tc_ctx.tile_pool(name="vsb", bufs=2))
        vdtpool = es.enter_context(tc_ctx.tile_pool(name="vdt", bufs=2))
        qpool = es.enter_context(tc_ctx.tile_pool(name="qld", bufs=2))
        ptpool = es.enter_context(tc_ctx.tile_pool(name="pt", bufs=4))
        cspool = es.enter_context(tc_ctx.tile_pool(name="cs", bufs=2))
        mpool = es.enter_context(tc_ctx.tile_pool(name="misc", bufs=2))
        stpool = es.enter_context(tc_ctx.tile_pool(name="agst", bufs=1))
        apool = es.enter_context(tc_ctx.tile_pool(name="ag", bufs=2))
        respool = es.enter_context(tc_ctx.tile_pool(name="res", bufs=2))
        # PSUM: 8 banks total.  acc (3, shared proj/o_proj) + scores (2,
        # also reused for the softmax-sum broadcast) + V-transpose (1) +
        # AV accumulator (2).
        ps_acc = es.enter_context(
            tc_ctx.tile_pool(name="ps_acc", bufs=3, space="PSUM"))
        ps_s = es.enter_context(
            tc_ctx.tile_pool(name="ps_s", bufs=2, space="PSUM"))
        ps_o = es.enter_context(
            tc_ctx.tile_pool(name="ps_o", bufs=2, space="PSUM"))
        ps_t = es.enter_context(
            tc_ctx.tile_pool(name="ps_t", bufs=1, space="PSUM"))

        wq_sb = wpool.tile([128, HL + 2, N_HT, 128], BF16)
        for ot in range(HL + 2):
            # Weights ride the scalar-engine HWDGE queue so the first X
            # chunk (sync queue) lands in parallel.
            nc.scalar.dma_start(out=wq_sb[:, ot], in_=wqkvt.ap()[:, ot])
        wo_sb = wopool.tile([128, N_HT, JC], BF16)
        nc.scalar.dma_start(out=wo_sb, in_=wot.ap())

        k_sb = {}                # per-batch [128, S] bf16 (d on partitions)
        v_sb = {}                # per-batch [128, n_kt, 128] bf16 ([t, d])

        # ---------------- QKV projection + RoPE (one chunk) -----------------
        def proj_chunk(ch):
            b = ch // (S // TC)
            cb = ch % (S // TC)          # chunk index within batch
            if cb == 0:
                k_sb[b] = kpool.tile([128, S], BF16, name=f"k_sb{b}")
                v_sb[b] = vpool.tile([128, n_kt, 128], BF16, name=f"v_sb{b}")
            xt_sb = xpool.tile([128, N_HT, TC], BF16)
            for hq in range(2):
                nc.sync.dma_start(
                    out=xt_sb[:, hq * 16:(hq + 1) * 16, :],
                    in_=xt.ap()[:, ch, hq * 16:(hq + 1) * 16, :],
                )
            rope_sb = rpool.tile([128, 2, TC], F32)
            nc.sync.dma_start(out=rope_sb, in_=ropes.ap()[:, ch])
            for ot in range(HL + 2):
                ps = ps_acc.tile([128, TC], F32, tag="acc")
                for h in range(N_HT):
                    nc.tensor.matmul(
                        ps,
                        lhsT=wq_sb[:, ot, h, :],
                        rhs=xt_sb[:, h, :],
                        start=(h == 0),
                        stop=(h == N_HT - 1),
                    )
                if ot < HL:
                    # RoPE for Q; bounce to HBM (qkt), reloaded per q-chunk.
                    sh = shpool.tile([128, TC], F32, tag="sh")
                    nc.vector.tensor_copy(sh[0:64, :], ps[64:128, :])
                    nc.vector.tensor_copy(sh[64:128, :], ps[0:64, :])
                    nc.vector.tensor_mul(sh, sh, rope_sb[:, 1, :])
                    tmp = shpool.tile([128, TC], F32, tag="tmp")
                    nc.vector.tensor_mul(tmp, ps, rope_sb[:, 0, :])
                    qk_out = opool.tile([128, TC], BF16, tag="qk")
                    nc.vector.tensor_add(qk_out, tmp, sh)
                    nc.sync.dma_start(
                        out=qkt[ot * 128:(ot + 1) * 128,
                                ch * TC:(ch + 1) * TC],
                        in_=qk_out,
                    )
                elif ot == HL:
                    # RoPE for K; written straight into the SBUF-resident
                    # per-batch K tile ([d, t] layout).
                    sh = shpool.tile([128, TC], F32, tag="sh")
                    nc.vector.tensor_copy(sh[0:64, :], ps[64:128, :])
                    nc.vector.tensor_copy(sh[64:128, :], ps[0:64, :])
                    nc.vector.tensor_mul(sh, sh, rope_sb[:, 1, :])
                    tmp = shpool.tile([128, TC], F32, tag="tmp")
                    nc.vector.tensor_mul(tmp, ps, rope_sb[:, 0, :])
                    nc.vector.tensor_add(
                        k_sb[b][:, cb * TC:(cb + 1) * TC], tmp, sh)
                else:
                    # V head: [d, t] -> PE-transpose 128x128 tiles -> [t, d].
                    v_dt = vdtpool.tile([128, TC], BF16)
                    nc.vector.tensor_copy(v_dt, ps)
                    pst = ps_t.tile([128, 2, 128], BF16, tag="pst")
                    for i in range(2):
                        nc.tensor.transpose(
                            pst[:, i, :], v_dt[:, i * 128:(i + 1) * 128],
                            ident)
                        nc.vector.tensor_copy(
                            v_sb[b][:, cb * 2 + i, :], pst[:, i, :])

        # ------------- attention q-chunk (4 local heads) + AG ---------------
        pace = {"inst": None}
        ags_fired = []
        oproj_done = []

        def attn_group(b, qc):
            chunk = b * N_QC + qc
            kt_max = 2 * (qc + 1)
            q_sbt = qpool.tile([128, HL, QC], BF16)
            nc.sync.dma_start(
                out=q_sbt,
                in_=qkt.rearrange("(hh p) t -> p hh t", p=128)[
                    :, 0:HL, b * S + qc * QC:b * S + (qc + 1) * QC],
            )
            ag_stage = stpool.tile([128, HL, QC], BF16)
            for hh in range(HL):
                pso = ps_o.tile([128, QC], F32, tag="pso")
                # Two parallel column-sum chains (even k-tiles on Vector,
                # odd on GpSimd) so neither engine's serial add chain paces
                # the softmax loop; merged by two accumulating ones-matmuls.
                cs_a = cspool.tile([128, QC], F32R, tag="csA")
                cs_b = cspool.tile([128, QC], F32R, tag="csB")
                for kt in range(kt_max):
                    d_off = kt - 2 * qc
                    ps = ps_s.tile([128, QC], F32, tag="s")
                    nc.tensor.matmul(
                        ps,
                        lhsT=k_sb[b][:, kt * 128:(kt + 1) * 128],
                        rhs=q_sbt[:, hh, :],
                        start=True,
                        stop=True,
                    )
                    pt = ptpool.tile([128, QC], BF16)
                    nc.scalar.activation(pt, ps, EXP, scale=SCALE)
                    if d_off >= 0:
                        nc.vector.tensor_mul(pt, pt, masks[:, d_off, :])
                    nc.tensor.matmul(
                        pso,
                        lhsT=v_sb[b][:, kt, :],
                        rhs=pt,
                        start=(kt == 0),
                        stop=(kt == kt_max - 1),
                    )
                    if kt == 0:
                        nc.vector.tensor_copy(cs_a, pt)
                    elif kt == 1:
                        nc.gpsimd.tensor_copy(cs_b, pt)
                    elif kt % 2 == 0:
                        nc.vector.tensor_add(cs_a, cs_a, pt)
                    else:
                        nc.gpsimd.tensor_add(cs_b, cs_b, pt)
                # Ones-matmuls both sum over k and broadcast the sums to all
                # partitions (reusing a scores slot).
                sums_bc = ps_s.tile([128, QC], F32, tag="s", name="sums_bc")
                nc.tensor.matmul(sums_bc, lhsT=ones_mat, rhs=cs_a,
                                 start=True, stop=(kt_max < 2))
                if kt_max >= 2:
                    nc.tensor.matmul(sums_bc, lhsT=ones_mat, rhs=cs_b,
                                     start=False, stop=True)
                recip_bc = mpool.tile([128, QC], F32, tag="recip_bc")
                rscr = mpool.tile([128, QC], F32, tag="rscr")
                nc.vector.reciprocal_approx_accurate(recip_bc, sums_bc, rscr)
                nc.vector.tensor_mul(ag_stage[:, hh, :], pso, recip_bc)
            wr = nc.sync.dma_start(out=ag_ins[chunk][:], in_=ag_stage)
            pace["inst"] = wr.ins
            nc.gpsimd.collective_compute(
                "AllGather",
                mybir.AluOpType.bypass,
                replica_groups=[list(range(N_CORES))],
                ins=[ag_ins[chunk][:]],
                outs=[ag_outs[chunk][:]],
            )
            ags_fired.append(chunk)

        # --------------------- o_proj (one AG chunk) ------------------------
        def emit_oproj(k):
            t0 = k * AGC
            ag_sbt = apool.tile([128, N_HT, AGC], BF16)
            ld = nc.scalar.dma_start(
                out=ag_sbt.rearrange("p (r hl) t -> p r hl t", r=N_CORES),
                in_=ag_outs[k].rearrange("r p hl t -> p r hl t"),
            )
            if pace["inst"] is not None:
                # Pace AllGather-output consumption behind real attention
                # progress; the scheduler's collective latency estimate is
                # optimistic and otherwise stalls the in-order PE stream.
                bass._add_dep_helper(
                    ld.ins, pace["inst"], sync=True,
                    reason="oproj paced behind attention",
                )
            for jt in range(JC // 128):
                ps4 = ps_acc.tile([128, AGC], F32, tag="acc", name="ps4")
                for h in range(N_HT):
                    nc.tensor.matmul(
                        ps4,
                        lhsT=wo_sb[:, h, jt * 128:(jt + 1) * 128],
                        rhs=ag_sbt[:, h, :],
                        start=(h == 0),
                        stop=(h == N_HT - 1),
                    )
                res4 = respool.tile([128, AGC], BF16)
                nc.vector.tensor_copy(res4, ps4)
                nc.scalar.dma_start(
                    out=out_t[jt * 128:(jt + 1) * 128, t0:t0 + AGC],
                    in_=res4,
                )
            oproj_done.append(k)

        # ------------------------- emission schedule ------------------------
        # P(ch): projection chunk; A(b,qc): attention q-chunk (needs proj
        # chunks 8b..8b+qc for K/V and 8b+qc for Q -> always >=2 chunks of
        # slack here).  o_proj k trails the AG stream by OPROJ_LAG.
        plan = [("P", 0), ("P", 1)]
        next_ch = 2
        for b in range(B):
            for qc in range(N_QC):
                plan.append(("A", b, qc))
                if next_ch < n_tc:
                    plan.append(("P", next_ch))
                    next_ch += 1
        # Reorder: keep P-chunks flowing a bit ahead of the A-groups they
        # feed.  plan above gives A(b,qc) after P(8b+qc+3).
        for item in plan:
            if item[0] == "P":
                proj_chunk(item[1])
            else:
                attn_group(item[1], item[2])
                if len(ags_fired) > OPROJ_LAG:
                    emit_oproj(ags_fired[-1 - OPROJ_LAG])
        for k in range(N_CH):
            if k not in oproj_done:
                emit_oproj(k)


def _build_program():
    nc = bacc.Bacc("TRN2", target_bir_lowering=False, debug=False,
                   num_devices=N_CORES)
    xt = nc.declare_dram_parameter("xt", [128, T // TC, N_HT, TC], BF16,
                                   isOutput=False)
    wqkvt = nc.declare_dram_parameter("wqkvt", [128, HL + 2, N_HT, 128], BF16,
                                      isOutput=False)
    wot = nc.declare_dram_parameter("wot", [128, N_HT, JC], BF16,
                                    isOutput=False)
    ropes = nc.declare_dram_parameter("ropes", [128, T // TC, 2, TC], F32,
                                      isOutput=False)
    out_t = nc.declare_dram_parameter("out_t", [JC, T], BF16, isOutput=True)

    qkt = nc.dram_tensor("qkt", [HL * D, T], BF16).ap()
    ag_ins = [nc.dram_tensor(f"ag_in{k}", [128, HL, AGC], BF16).ap()
              for k in range(N_CH)]
    ag_outs = [nc.dram_tensor(f"ag_out{k}", [N_CORES, 128, HL, AGC], BF16,
                              addr_space="Shared").ap()
               for k in range(N_CH)]

    with tile.TileContext(nc) as tc_ctx:
        _emit(tc_ctx, xt, wqkvt, wot, ropes, out_t, qkt, ag_ins, ag_outs)
    nc.finalize()
    return nc


def _host_inputs(hidden_states, w_qkv, w_o):
    """Shard + transpose inputs for the 8 cores; returns in_maps."""
    X = np.asarray(hidden_states, dtype=np.float32).reshape(T, HID)
    # [p, ch, ht, tc] tiled layout so every DMA line is contiguous.
    xt = np.ascontiguousarray(
        X.reshape(T // TC, TC, N_HT, 128).transpose(3, 0, 2, 1)
    ).astype(ml_dtypes.bfloat16)

    # RoPE tables in [d, t] layout with rotate-half sign folded into sin and
    # the attention scale folded into the Q tables.
    inv_freq = 1.0 / (ROPE_BASE ** (np.arange(0, D, 2, dtype=np.float32) / D))
    pos = np.arange(S, dtype=np.float32)
    freqs = np.outer(pos, inv_freq)                      # (S, D/2)
    emb = np.concatenate([freqs, freqs], axis=-1)        # (S, D)
    cos = np.cos(emb).T.astype(np.float32)               # (D, S)
    sin = np.sin(emb).T.astype(np.float32)
    sgn = np.concatenate([-np.ones(D // 2), np.ones(D // 2)]).astype(np.float32)
    sins = sgn[:, None] * sin
    cos_t = np.tile(cos, (1, B))                         # (D, T)
    sins_t = np.tile(sins, (1, B))
    ropes = np.stack([cos_t, sins_t], axis=0)
    ropes = np.ascontiguousarray(
        ropes.reshape(2, 128, T // TC, TC).transpose(1, 2, 0, 3),
        dtype=np.float32,
    )

    w_qkv = np.asarray(w_qkv, dtype=np.float32)
    w_o = np.asarray(w_o, dtype=np.float32)
    q_sz = N_HEADS * D
    kv_sz = N_KV_HEADS * D
    in_maps = []
    for c in range(N_CORES):
        qr = w_qkv[c * HL * D:(c + 1) * HL * D]
        kr = w_qkv[q_sz + c * D:q_sz + (c + 1) * D]
        vr = w_qkv[q_sz + kv_sz + c * D:q_sz + kv_sz + (c + 1) * D]
        w_shard = np.concatenate([qr, kr, vr], axis=0)           # (768, HID)
        wqkvt_c = np.ascontiguousarray(
            w_shard.reshape(HL + 2, 128, N_HT, 128).transpose(3, 0, 2, 1)
        ).astype(ml_dtypes.bfloat16)
        wot_c = np.ascontiguousarray(
            w_o[c * JC:(c + 1) * JC, :].reshape(JC, N_HT, 128).transpose(2, 1, 0)
        ).astype(ml_dtypes.bfloat16)
        in_maps.append({
            "xt": xt, "wqkvt": wqkvt_c, "wot": wot_c, "ropes": ropes,
        })
    return in_maps


def _run(hidden_states, w_qkv, w_o, trace=False, tmpdir=None):
    in_maps = _host_inputs(hidden_states, w_qkv, w_o)
    nc = _build_program()
    res = run_bass_kernel_spmd(nc, in_maps, list(range(N_CORES)),
                               trace=trace, tmpdir=tmpdir)
    out_T = np.concatenate(
        [np.asarray(res.results[c]["out_t"]).astype(np.float32)
         for c in range(N_CORES)], axis=0
    )                                                     # (HID j, T)
    out = np.ascontiguousarray(out_T.T).reshape(B, S, HID).astype(np.float32)
    return out, res


def kernel(hidden_states, w_qkv, w_o):
    out, _ = _run(hidden_states, w_qkv, w_o, trace=False)
    return out


# revision 11
# speedup vs baseline: 1.2674x; 1.2207x over previous
"""Trainium2 Bass kernel: dense transformer attention block (QKV proj + RoPE +
GQA causal attention + output proj), tensor-parallel over 8 NeuronCores.

Sharding: heads are split across cores (4 Q heads + 1 KV head per core).
Single fully-interleaved pipeline: QKV-projection token chunks, attention
q-chunks (forward order, so chunk qc only needs K/V through token
256*(qc+1)), per-chunk AllGathers of the head-sharded attention output, and
o_proj column-slice chunks are all emitted interleaved so the Tile
scheduler can pack the TensorEngine with projection matmuls inside the
softmax (scalar-EXP) latency gaps and spread the AllGather stream across
the whole kernel instead of serializing it at the end.  K and V^T live in
SBUF (V transposed on the PE via identity-matmul); only Q bounces through
HBM.  Softmax column-sum accumulation runs on the (otherwise idle) GpSimd
engine.
"""

from contextlib import ExitStack

import numpy as np
import ml_dtypes

import concourse.bass as bass
from concourse import bacc
import concourse.tile as tile
import concourse.mybir as mybir
from concourse.bass_utils import run_bass_kernel_spmd

F32 = mybir.dt.float32
F32R = mybir.dt.float32r
BF16 = mybir.dt.bfloat16
EXP = mybir.ActivationFunctionType.Exp
SCALE = float(128 ** -0.5)

N_CORES = 8
N_HEADS = 32
N_KV_HEADS = 8
D = 128          # head dim
HID = 4096
B = 2
S = 2048
T = B * S        # 4096 tokens
ROPE_BASE = 10000.0

HL = N_HEADS // N_CORES          # 4 local Q heads per core
QKV_ROWS = (HL + 2) * D          # 768: 4 Q heads + 1 K head + 1 V head
JC = HID // N_CORES              # 512 output columns per core

TC = 256                         # token chunk for the QKV projection phase
QC = 256                         # query chunk in attention
N_HT = HID // 128                # 32 hidden tiles
N_QC = S // QC                   # 8 q-chunks per batch
AGC = 256                        # AllGather chunk granularity (tokens)
N_CH = T // AGC                  # 16 AllGather chunks
OPROJ_LAG = 3                    # o_proj trails the AG stream by this many


def _emit(tc_ctx, xt, wqkvt, wot, ropes, out_t, qkt, ag_ins, ag_outs):
    nc = tc_ctx.nc
    n_tc = T // TC           # 16 projection chunks
    n_kt = S // 128          # 16 k-tiles per batch

    with ExitStack() as es:
        const_pool = es.enter_context(tc_ctx.tile_pool(name="const", bufs=1))
        # All-ones stationary: one matmul computes column sums AND broadcasts
        # them across all 128 partitions.
        ones_mat = const_pool.tile([128, 128], F32R)
        # Identity (bf16) for PE-transposing V tiles.
        ident = const_pool.tile([128, 128], BF16)
        # Diagonal causal masks: mask_d[k, q] = 1.0 if q - k - 128*d >= 0.
        masks = const_pool.tile([128, 2, QC], BF16)
        nc.vector.memset(masks, 1.0)
        nc.vector.tensor_copy(ones_mat, masks[:, 0, 0:128])
        nc.vector.tensor_copy(ident, masks[:, 0, 0:128])
        nc.gpsimd.affine_select(
            out=ident, in_=ident,
            compare_op=mybir.AluOpType.is_equal,
            fill=0.0, base=0, pattern=[[1, 128]], channel_multiplier=-1,
        )
        for d_off in range(2):
            nc.gpsimd.affine_select(
                out=masks[:, d_off, :],
                in_=masks[:, d_off, :],
                compare_op=mybir.AluOpType.is_ge,
                fill=0.0,
                base=-128 * d_off,
                pattern=[[1, QC]],
                channel_multiplier=-1,
            )

        # ------------------------- persistent pools -------------------------
        wpool = es.enter_context(tc_ctx.tile_pool(name="wq", bufs=1))
        wopool = es.enter_context(tc_ctx.tile_pool(name="wo", bufs=1))
        xpool = es.enter_context(tc_ctx.tile_pool(name="x", bufs=2))
        rpool = es.enter_context(tc_ctx.tile_pool(name="rope", bufs=1))
        shpool = es.enter_context(tc_ctx.tile_pool(name="rsh", bufs=2))
        opool = es.enter_context(tc_ctx.tile_pool(name="p1out", bufs=2))
        kpool = es.enter_context(tc_ctx.tile_pool(name="ksb", bufs=2))
        vpool = es.enter_context(tc_ctx.tile_pool(name="vsb", bufs=2))
        vdtpool = es.enter_context(tc_ctx.tile_pool(name="vdt", bufs=1))
        qpool = es.enter_context(tc_ctx.tile_pool(name="qld", bufs=2))
        ptpool = es.enter_context(tc_ctx.tile_pool(name="pt", bufs=3))
        cspool = es.enter_context(tc_ctx.tile_pool(name="cs", bufs=2))
        mpool = es.enter_context(tc_ctx.tile_pool(name="misc", bufs=1))
        stpool = es.enter_context(tc_ctx.tile_pool(name="agst", bufs=2))
        apool = es.enter_context(tc_ctx.tile_pool(name="ag", bufs=2))
        respool = es.enter_context(tc_ctx.tile_pool(name="res", bufs=2))
        # PSUM: 8 banks total.  acc (3, shared proj/o_proj) + scores (2,
        # also reused for the softmax-sum broadcast) + V-transpose (1) +
        # AV accumulator (2).
        ps_acc = es.enter_context(
            tc_ctx.tile_pool(name="ps_acc", bufs=3, space="PSUM"))
        ps_s = es.enter_context(
            tc_ctx.tile_pool(name="ps_s", bufs=2, space="PSUM"))
        ps_o = es.enter_context(
            tc_ctx.tile_pool(name="ps_o", bufs=2, space="PSUM"))
        ps_t = es.enter_context(
            tc_ctx.tile_pool(name="ps_t", bufs=1, space="PSUM"))

        wq_sb = wpool.tile([128, HL + 2, N_HT, 128], BF16)
        for ot in range(HL + 2):
            # Weights ride the scalar-engine HWDGE queue so the first X
            # chunk (sync queue) lands in parallel.
            nc.scalar.dma_start(out=wq_sb[:, ot], in_=wqkvt.ap()[:, ot])
        wo_sb = wopool.tile([128, N_HT, JC], BF16)
        nc.scalar.dma_start(out=wo_sb, in_=wot.ap())

        k_sb = {}                # per-batch [128, S] bf16 (d on partitions)
        v_sb = {}                # per-batch [128, n_kt, 128] bf16 ([t, d])

        # ---------------- QKV projection + RoPE (one chunk) -----------------
        def proj_chunk(ch):
            b = ch // (S // TC)
            cb = ch % (S // TC)          # chunk index within batch
            if cb == 0:
                k_sb[b] = kpool.tile([128, S], BF16, name=f"k_sb{b}")
                v_sb[b] = vpool.tile([128, n_kt, 128], BF16, name=f"v_sb{b}")
            xt_sb = xpool.tile([128, N_HT, TC], BF16)
            for hq in range(2):
                nc.sync.dma_start(
                    out=xt_sb[:, hq * 16:(hq + 1) * 16, :],
                    in_=xt.ap()[:, ch, hq * 16:(hq + 1) * 16, :],
                )
            rope_sb = rpool.tile([128, 2, TC], F32)
            nc.sync.dma_start(out=rope_sb, in_=ropes.ap()[:, ch])
            for ot in range(HL + 2):
                ps = ps_acc.tile([128, TC], F32, tag="acc")
                for h in range(N_HT):
                    nc.tensor.matmul(
                        ps,
                        lhsT=wq_sb[:, ot, h, :],
                        rhs=xt_sb[:, h, :],
                        start=(h == 0),
                        stop=(h == N_HT - 1),
                    )
                if ot < HL:
                    # RoPE for Q; bounce to HBM (qkt), reloaded per q-chunk.
                    sh = shpool.tile([128, TC], F32, tag="sh")
                    nc.vector.tensor_copy(sh[0:64, :], ps[64:128, :])
                    nc.vector.tensor_copy(sh[64:128, :], ps[0:64, :])
                    nc.vector.tensor_mul(sh, sh, rope_sb[:, 1, :])
                    tmp = shpool.tile([128, TC], F32, tag="tmp")
                    nc.vector.tensor_mul(tmp, ps, rope_sb[:, 0, :])
                    qk_out = opool.tile([128, TC], BF16, tag="qk")
                    nc.vector.tensor_add(qk_out, tmp, sh)
                    nc.sync.dma_start(
                        out=qkt[ot * 128:(ot + 1) * 128,
                                ch * TC:(ch + 1) * TC],
                        in_=qk_out,
                    )
                elif ot == HL:
                    # RoPE for K; written straight into the SBUF-resident
                    # per-batch K tile ([d, t] layout).
                    sh = shpool.tile([128, TC], F32, tag="sh")
                    nc.vector.tensor_copy(sh[0:64, :], ps[64:128, :])
                    nc.vector.tensor_copy(sh[64:128, :], ps[0:64, :])
                    nc.vector.tensor_mul(sh, sh, rope_sb[:, 1, :])
                    tmp = shpool.tile([128, TC], F32, tag="tmp")
                    nc.vector.tensor_mul(tmp, ps, rope_sb[:, 0, :])
                    nc.vector.tensor_add(
                        k_sb[b][:, cb * TC:(cb + 1) * TC], tmp, sh)
                else:
                    # V head: [d, t] -> PE-transpose 128x128 tiles -> [t, d].
                    v_dt = vdtpool.tile([128, TC], BF16)
                    nc.vector.tensor_copy(v_dt, ps)
                    pst = ps_t.tile([128, 2, 128], BF16, tag="pst")
                    for i in range(2):
                        nc.tensor.transpose(
                            pst[:, i, :], v_dt[:, i * 128:(i + 1) * 128],
                            ident)
                        nc.vector.tensor_copy(
                            v_sb[b][:, cb * 2 + i, :], pst[:, i, :])

        # ------------- attention q-chunk (4 local heads) + AG ---------------
        pace = {"inst": None}
        ags_fired = []
        oproj_done = []

        def attn_group(b, qc):
            chunk = b * N_QC + qc
            kt_max = 2 * (qc + 1)
            q_sbt = qpool.tile([128, HL, QC], BF16)
            nc.sync.dma_start(
                out=q_sbt,
                in_=qkt.rearrange("(hh p) t -> p hh t", p=128)[
                    :, 0:HL, b * S + qc * QC:b * S + (qc + 1) * QC],
            )
            ag_stage = stpool.tile([128, HL, QC], BF16)
            for hh in range(HL):
                pso = ps_o.tile([128, QC], F32, tag="pso")
                # Two parallel column-sum chains (even k-tiles on Vector,
                # odd on GpSimd) so neither engine's serial add chain paces
                # the softmax loop; merged by two accumulating ones-matmuls.
                cs_a = cspool.tile([128, QC], F32R, tag="csA")
                cs_b = cspool.tile([128, QC], F32R, tag="csB")
                for kt in range(kt_max):
                    d_off = kt - 2 * qc
                    ps = ps_s.tile([128, QC], F32, tag="s")
                    nc.tensor.matmul(
                        ps,
                        lhsT=k_sb[b][:, kt * 128:(kt + 1) * 128],
                        rhs=q_sbt[:, hh, :],
                        start=True,
                        stop=True,
                    )
                    pt = ptpool.tile([128, QC], BF16)
                    nc.scalar.activation(pt, ps, EXP, scale=SCALE)
                    if d_off >= 0:
                        nc.vector.tensor_mul(pt, pt, masks[:, d_off, :])
                    nc.tensor.matmul(
                        pso,
                        lhsT=v_sb[b][:, kt, :],
                        rhs=pt,
                        start=(kt == 0),
                        stop=(kt == kt_max - 1),
                    )
                    if kt == 0:
                        nc.vector.tensor_copy(cs_a, pt)
                    elif kt == 1:
                        nc.gpsimd.tensor_copy(cs_b, pt)
                    elif kt % 2 == 0:
                        nc.vector.tensor_add(cs_a, cs_a, pt)
                    else:
                        nc.gpsimd.tensor_add(cs_b, cs_b, pt)
                # Ones-matmuls both sum over k and broadcast the sums to all
                # partitions (reusing a scores slot).
                sums_bc = ps_s.tile([128, QC], F32, tag="s", name="sums_bc")
                nc.tensor.matmul(sums_bc, lhsT=ones_mat, rhs=cs_a,
                                 start=True, stop=(kt_max < 2))
                if kt_max >= 2:
                    nc.tensor.matmul(sums_bc, lhsT=ones_mat, rhs=cs_b,
                                     start=False, stop=True)
                recip_bc = mpool.tile([128, QC], F32, tag="recip_bc")
                rscr = mpool.tile([128, QC], F32, tag="rscr")
                nc.vector.reciprocal_approx_accurate(recip_bc, sums_bc, rscr)
                nc.vector.tensor_mul(ag_stage[:, hh, :], pso, recip_bc)
            wr = nc.sync.dma_start(out=ag_ins[chunk][:], in_=ag_stage)
            pace["inst"] = wr.ins
            nc.gpsimd.collective_compute(
                "AllGather",
                mybir.AluOpType.bypass,
                replica_groups=[list(range(N_CORES))],
                ins=[ag_ins[chunk][:]],
                outs=[ag_outs[chunk][:]],
            )
            ags_fired.append(chunk)

        # --------------------- o_proj (one AG chunk) ------------------------
        # The AG-output load is emitted one attention chunk ahead of its
        # matmuls so the 2 MB DMA overlaps compute instead of bubbling the
        # in-order PE stream.
        ag_tiles = {}

        def emit_oproj_load(k):
            ag_sbt = apool.tile([128, N_HT, AGC], BF16)
            ld = nc.scalar.dma_start(
                out=ag_sbt.rearrange("p (r hl) t -> p r hl t", r=N_CORES),
                in_=ag_outs[k].rearrange("r p hl t -> p r hl t"),
            )
            if pace["inst"] is not None:
                # Pace AllGather-output consumption behind real attention
                # progress; the scheduler's collective latency estimate is
                # optimistic and would otherwise hoist this load (and its
                # dependent matmuls) too early in the PE stream.
                bass._add_dep_helper(
                    ld.ins, pace["inst"], sync=True,
                    reason="oproj paced behind attention",
                )
            ag_tiles[k] = ag_sbt

        def emit_oproj_mm(k):
            t0 = k * AGC
            ag_sbt = ag_tiles.pop(k)
            for jt in range(JC // 128):
                ps4 = ps_acc.tile([128, AGC], F32, tag="acc", name="ps4")
                for h in range(N_HT):
                    nc.tensor.matmul(
                        ps4,
                        lhsT=wo_sb[:, h, jt * 128:(jt + 1) * 128],
                        rhs=ag_sbt[:, h, :],
                        start=(h == 0),
                        stop=(h == N_HT - 1),
                    )
                res4 = respool.tile([128, AGC], BF16)
                nc.vector.tensor_copy(res4, ps4)
                nc.scalar.dma_start(
                    out=out_t[jt * 128:(jt + 1) * 128, t0:t0 + AGC],
                    in_=res4,
                )
            oproj_done.append(k)

        # ------------------------- emission schedule ------------------------
        # P(ch): projection chunk; A(b,qc): attention q-chunk (needs proj
        # chunks 8b..8b+qc for K/V and 8b+qc for Q -> always >=2 chunks of
        # slack here).  o_proj k trails the AG stream by OPROJ_LAG.
        plan = [("P", 0), ("P", 1)]
        next_ch = 2
        for b in range(B):
            for qc in range(N_QC):
                plan.append(("A", b, qc))
                if next_ch < n_tc:
                    plan.append(("P", next_ch))
                    next_ch += 1
        # Reorder: keep P-chunks flowing a bit ahead of the A-groups they
        # feed.  plan above gives A(b,qc) after P(8b+qc+3).
        for item in plan:
            if item[0] == "P":
                proj_chunk(item[1])
            else:
                attn_group(item[1], item[2])
                n = len(ags_fired)
                if n > OPROJ_LAG:
                    emit_oproj_load(ags_fired[-1 - OPROJ_LAG])
                if n > OPROJ_LAG + 1:
                    emit_oproj_mm(ags_fired[-2 - OPROJ_LAG])
        for k in range(N_CH):
            if k not in ag_tiles and k not in oproj_done:
                emit_oproj_load(k)
                emit_oproj_mm(k)
            elif k in ag_tiles:
                emit_oproj_mm(k)


def _build_program():
    nc = bacc.Bacc("TRN2", target_bir_lowering=False, debug=False,
                   num_devices=N_CORES)
    xt = nc.declare_dram_parameter("xt", [128, T // TC, N_HT, TC], BF16,
                                   isOutput=False)
    wqkvt = nc.declare_dram_parameter("wqkvt", [128, HL + 2, N_HT, 128], BF16,
                                      isOutput=False)
    wot = nc.declare_dram_parameter("wot", [128, N_HT, JC], BF16,
                                    isOutput=False)
    ropes = nc.declare_dram_parameter("ropes", [128, T // TC, 2, TC], F32,
                                      isOutput=False)
    out_t = nc.declare_dram_parameter("out_t", [JC, T], BF16, isOutput=True)

    qkt = nc.dram_tensor("qkt", [HL * D, T], BF16).ap()
    ag_ins = [nc.dram_tensor(f"ag_in{k}", [128, HL, AGC], BF16).ap()
              for k in range(N_CH)]
    ag_outs = [nc.dram_tensor(f"ag_out{k}", [N_CORES, 128, HL, AGC], BF16,
                              addr_space="Shared").ap()
               for k in range(N_CH)]

    with tile.TileContext(nc) as tc_ctx:
        _emit(tc_ctx, xt, wqkvt, wot, ropes, out_t, qkt, ag_ins, ag_outs)
    nc.finalize()
    return nc


def _host_inputs(hidden_states, w_qkv, w_o):
    """Shard + transpose inputs for the 8 cores; returns in_maps."""
    X = np.asarray(hidden_states, dtype=np.float32).reshape(T, HID)
    # [p, ch, ht, tc] tiled layout so every DMA line is contiguous.
    xt = np.ascontiguousarray(
        X.reshape(T // TC, TC, N_HT, 128).transpose(3, 0, 2, 1)
    ).astype(ml_dtypes.bfloat16)

    # RoPE tables in [d, t] layout with rotate-half sign folded into sin and
    # the attention scale folded into the Q tables.
    inv_freq = 1.0 / (ROPE_BASE ** (np.arange(0, D, 2, dtype=np.float32) / D))
    pos = np.arange(S, dtype=np.float32)
    freqs = np.outer(pos, inv_freq)                      # (S, D/2)
    emb = np.concatenate([freqs, freqs], axis=-1)        # (S, D)
    cos = np.cos(emb).T.astype(np.float32)               # (D, S)
    sin = np.sin(emb).T.astype(np.float32)
    sgn = np.concatenate([-np.ones(D // 2), np.ones(D // 2)]).astype(np.float32)
    sins = sgn[:, None] * sin
    cos_t = np.tile(cos, (1, B))                         # (D, T)
    sins_t = np.tile(sins, (1, B))
    ropes = np.stack([cos_t, sins_t], axis=0)
    ropes = np.ascontiguousarray(
        ropes.reshape(2, 128, T // TC, TC).transpose(1, 2, 0, 3),
        dtype=np.float32,
    )

    w_qkv = np.asarray(w_qkv, dtype=np.float32)
    w_o = np.asarray(w_o, dtype=np.float32)
    q_sz = N_HEADS * D
    kv_sz = N_KV_HEADS * D
    in_maps = []
    for c in range(N_CORES):
        qr = w_qkv[c * HL * D:(c + 1) * HL * D]
        kr = w_qkv[q_sz + c * D:q_sz + (c + 1) * D]
        vr = w_qkv[q_sz + kv_sz + c * D:q_sz + kv_sz + (c + 1) * D]
        w_shard = np.concatenate([qr, kr, vr], axis=0)           # (768, HID)
        wqkvt_c = np.ascontiguousarray(
            w_shard.reshape(HL + 2, 128, N_HT, 128).transpose(3, 0, 2, 1)
        ).astype(ml_dtypes.bfloat16)
        wot_c = np.ascontiguousarray(
            w_o[c * JC:(c + 1) * JC, :].reshape(JC, N_HT, 128).transpose(2, 1, 0)
        ).astype(ml_dtypes.bfloat16)
        in_maps.append({
            "xt": xt, "wqkvt": wqkvt_c, "wot": wot_c, "ropes": ropes,
        })
    return in_maps


def _run(hidden_states, w_qkv, w_o, trace=False, tmpdir=None):
    in_maps = _host_inputs(hidden_states, w_qkv, w_o)
    nc = _build_program()
    res = run_bass_kernel_spmd(nc, in_maps, list(range(N_CORES)),
                               trace=trace, tmpdir=tmpdir)
    out_T = np.concatenate(
        [np.asarray(res.results[c]["out_t"]).astype(np.float32)
         for c in range(N_CORES)], axis=0
    )                                                     # (HID j, T)
    out = np.ascontiguousarray(out_T.T).reshape(B, S, HID).astype(np.float32)
    return out, res


def kernel(hidden_states, w_qkv, w_o):
    out, _ = _run(hidden_states, w_qkv, w_o, trace=False)
    return out


# revision 13
# speedup vs baseline: 1.2787x; 1.0089x over previous
"""Trainium2 Bass kernel: dense transformer attention block (QKV proj + RoPE +
GQA causal attention + output proj), tensor-parallel over 8 NeuronCores.

Sharding: heads are split across cores (4 Q heads + 1 KV head per core).
Single fully-interleaved pipeline: QKV-projection token chunks, attention
q-chunks (forward order, so chunk qc only needs K/V through token
256*(qc+1)), per-chunk AllGathers of the head-sharded attention output, and
o_proj column-slice chunks are all emitted interleaved so the Tile
scheduler can pack the TensorEngine with projection matmuls inside the
softmax (scalar-EXP) latency gaps and spread the AllGather stream across
the whole kernel instead of serializing it at the end.  K and V^T live in
SBUF (V transposed on the PE via identity-matmul); only Q bounces through
HBM.  Softmax column-sum accumulation runs on the (otherwise idle) GpSimd
engine.
"""

from contextlib import ExitStack

import numpy as np
import ml_dtypes

import concourse.bass as bass
from concourse import bacc
import concourse.tile as tile
import concourse.mybir as mybir
from concourse.bass_utils import run_bass_kernel_spmd

F32 = mybir.dt.float32
F32R = mybir.dt.float32r
BF16 = mybir.dt.bfloat16
EXP = mybir.ActivationFunctionType.Exp
SCALE = float(128 ** -0.5)

N_CORES = 8
N_HEADS = 32
N_KV_HEADS = 8
D = 128          # head dim
HID = 4096
B = 2
S = 2048
T = B * S        # 4096 tokens
ROPE_BASE = 10000.0

HL = N_HEADS // N_CORES          # 4 local Q heads per core
QKV_ROWS = (HL + 2) * D          # 768: 4 Q heads + 1 K head + 1 V head
JC = HID // N_CORES              # 512 output columns per core

TC = 256                         # token chunk for the QKV projection phase
QC = 256                         # query chunk in attention
N_HT = HID // 128                # 32 hidden tiles
N_QC = S // QC                   # 8 q-chunks per batch
AGC = 256                        # AllGather chunk granularity (tokens)
N_CH = T // AGC                  # 16 AllGather chunks
OPROJ_LAG = 3                    # o_proj trails the AG stream by this many


def _emit(tc_ctx, xt, wqkvt, wot, ropes, out_t, qkt, ag_ins, ag_outs):
    nc = tc_ctx.nc
    n_tc = T // TC           # 16 projection chunks
    n_kt = S // 128          # 16 k-tiles per batch

    with ExitStack() as es:
        const_pool = es.enter_context(tc_ctx.tile_pool(name="const", bufs=1))
        # All-ones stationary: one matmul computes column sums AND broadcasts
        # them across all 128 partitions.
        ones_mat = const_pool.tile([128, 128], F32R)
        # Identity (bf16) for PE-transposing V tiles.
        ident = const_pool.tile([128, 128], BF16)
        # Diagonal causal masks: mask_d[k, q] = 1.0 if q - k - 128*d >= 0.
        masks = const_pool.tile([128, 2, QC], BF16)
        nc.vector.memset(masks, 1.0)
        nc.vector.tensor_copy(ones_mat, masks[:, 0, 0:128])
        nc.vector.tensor_copy(ident, masks[:, 0, 0:128])
        nc.gpsimd.affine_select(
            out=ident, in_=ident,
            compare_op=mybir.AluOpType.is_equal,
            fill=0.0, base=0, pattern=[[1, 128]], channel_multiplier=-1,
        )
        for d_off in range(2):
            nc.gpsimd.affine_select(
                out=masks[:, d_off, :],
                in_=masks[:, d_off, :],
                compare_op=mybir.AluOpType.is_ge,
                fill=0.0,
                base=-128 * d_off,
                pattern=[[1, QC]],
                channel_multiplier=-1,
            )

        # ------------------------- persistent pools -------------------------
        wpool = es.enter_context(tc_ctx.tile_pool(name="wq", bufs=1))
        wopool = es.enter_context(tc_ctx.tile_pool(name="wo", bufs=1))
        xpool = es.enter_context(tc_ctx.tile_pool(name="x", bufs=2))
        rpool = es.enter_context(tc_ctx.tile_pool(name="rope", bufs=1))
        shpool = es.enter_context(tc_ctx.tile_pool(name="rsh", bufs=2))
        opool = es.enter_context(tc_ctx.tile_pool(name="p1out", bufs=2))
        kpool = es.enter_context(tc_ctx.tile_pool(name="ksb", bufs=2))
        vpool = es.enter_context(tc_ctx.tile_pool(name="vsb", bufs=2))
        vdtpool = es.enter_context(tc_ctx.tile_pool(name="vdt", bufs=1))
        qpool = es.enter_context(tc_ctx.tile_pool(name="qld", bufs=2))
        ptpool = es.enter_context(tc_ctx.tile_pool(name="pt", bufs=3))
        cspool = es.enter_context(tc_ctx.tile_pool(name="cs", bufs=2))
        mpool = es.enter_context(tc_ctx.tile_pool(name="misc", bufs=1))
        stpool = es.enter_context(tc_ctx.tile_pool(name="agst", bufs=2))
        apool = es.enter_context(tc_ctx.tile_pool(name="ag", bufs=2))
        respool = es.enter_context(tc_ctx.tile_pool(name="res", bufs=3))
        # PSUM: 8 banks total.  acc (3, shared proj/o_proj) + scores (2,
        # also reused for the softmax-sum broadcast) + V-transpose (1) +
        # AV accumulator (2).
        ps_acc = es.enter_context(
            tc_ctx.tile_pool(name="ps_acc", bufs=3, space="PSUM"))
        ps_s = es.enter_context(
            tc_ctx.tile_pool(name="ps_s", bufs=2, space="PSUM"))
        ps_o = es.enter_context(
            tc_ctx.tile_pool(name="ps_o", bufs=2, space="PSUM"))
        ps_t = es.enter_context(
            tc_ctx.tile_pool(name="ps_t", bufs=1, space="PSUM"))

        wq_sb = wpool.tile([128, HL + 2, N_HT, 128], BF16)
        for ot in range(HL + 2):
            # Weights ride the scalar-engine HWDGE queue so the first X
            # chunk (sync queue) lands in parallel.
            nc.scalar.dma_start(out=wq_sb[:, ot], in_=wqkvt.ap()[:, ot])
        wo_sb = wopool.tile([128, N_HT, JC], BF16)
        nc.scalar.dma_start(out=wo_sb, in_=wot.ap())

        k_sb = {}                # per-batch [128, S] bf16 (d on partitions)
        v_sb = {}                # per-batch [128, n_kt, 128] bf16 ([t, d])

        # ---------------- QKV projection + RoPE (one chunk) -----------------
        def proj_chunk(ch):
            b = ch // (S // TC)
            cb = ch % (S // TC)          # chunk index within batch
            if cb == 0:
                k_sb[b] = kpool.tile([128, S], BF16, name=f"k_sb{b}")
                v_sb[b] = vpool.tile([128, n_kt, 128], BF16, name=f"v_sb{b}")
            xt_sb = xpool.tile([128, N_HT, TC], BF16)
            for hq in range(2):
                nc.sync.dma_start(
                    out=xt_sb[:, hq * 16:(hq + 1) * 16, :],
                    in_=xt.ap()[:, ch, hq * 16:(hq + 1) * 16, :],
                )
            rope_sb = rpool.tile([128, 2, TC], F32)
            nc.sync.dma_start(out=rope_sb, in_=ropes.ap()[:, ch])
            for ot in range(HL + 2):
                ps = ps_acc.tile([128, TC], F32, tag="acc")
                for h in range(N_HT):
                    nc.tensor.matmul(
                        ps,
                        lhsT=wq_sb[:, ot, h, :],
                        rhs=xt_sb[:, h, :],
                        start=(h == 0),
                        stop=(h == N_HT - 1),
                    )
                if ot < HL:
                    # RoPE for Q; bounce to HBM (qkt), reloaded per q-chunk.
                    sh = shpool.tile([128, TC], F32, tag="sh")
                    nc.vector.tensor_copy(sh[0:64, :], ps[64:128, :])
                    nc.vector.tensor_copy(sh[64:128, :], ps[0:64, :])
                    nc.vector.tensor_mul(sh, sh, rope_sb[:, 1, :])
                    tmp = shpool.tile([128, TC], F32, tag="tmp")
                    nc.vector.tensor_mul(tmp, ps, rope_sb[:, 0, :])
                    qk_out = opool.tile([128, TC], BF16, tag="qk")
                    nc.vector.tensor_add(qk_out, tmp, sh)
                    nc.sync.dma_start(
                        out=qkt[ot * 128:(ot + 1) * 128,
                                ch * TC:(ch + 1) * TC],
                        in_=qk_out,
                    )
                elif ot == HL:
                    # RoPE for K; written straight into the SBUF-resident
                    # per-batch K tile ([d, t] layout).
                    sh = shpool.tile([128, TC], F32, tag="sh")
                    nc.vector.tensor_copy(sh[0:64, :], ps[64:128, :])
                    nc.vector.tensor_copy(sh[64:128, :], ps[0:64, :])
                    nc.vector.tensor_mul(sh, sh, rope_sb[:, 1, :])
                    tmp = shpool.tile([128, TC], F32, tag="tmp")
                    nc.vector.tensor_mul(tmp, ps, rope_sb[:, 0, :])
                    nc.vector.tensor_add(
                        k_sb[b][:, cb * TC:(cb + 1) * TC], tmp, sh)
                else:
                    # V head: [d, t] -> PE-transpose 128x128 tiles -> [t, d].
                    v_dt = vdtpool.tile([128, TC], BF16)
                    nc.vector.tensor_copy(v_dt, ps)
                    pst = ps_t.tile([128, 2, 128], BF16, tag="pst")
                    for i in range(2):
                        nc.tensor.transpose(
                            pst[:, i, :], v_dt[:, i * 128:(i + 1) * 128],
                            ident)
                        nc.vector.tensor_copy(
                            v_sb[b][:, cb * 2 + i, :], pst[:, i, :])

        # ------------- attention q-chunk (4 local heads) + AG ---------------
        pace = {"inst": None}
        ags_fired = []
        oproj_done = []

        def attn_group(b, qc):
            chunk = b * N_QC + qc
            kt_max = 2 * (qc + 1)
            q_sbt = qpool.tile([128, HL, QC], BF16)
            nc.sync.dma_start(
                out=q_sbt,
                in_=qkt.rearrange("(hh p) t -> p hh t", p=128)[
                    :, 0:HL, b * S + qc * QC:b * S + (qc + 1) * QC],
            )
            ag_stage = stpool.tile([128, HL, QC], BF16)
            for hh in range(HL):
                pso = ps_o.tile([128, QC], F32, tag="pso")
                # Two parallel column-sum chains (even k-tiles on Vector,
                # odd on GpSimd) so neither engine's serial add chain paces
                # the softmax loop; merged by two accumulating ones-matmuls.
                cs_a = cspool.tile([128, QC], F32R, tag="csA")
                cs_b = cspool.tile([128, QC], F32R, tag="csB")
                for kt in range(kt_max):
                    d_off = kt - 2 * qc
                    ps = ps_s.tile([128, QC], F32, tag="s")
                    nc.tensor.matmul(
                        ps,
                        lhsT=k_sb[b][:, kt * 128:(kt + 1) * 128],
                        rhs=q_sbt[:, hh, :],
                        start=True,
                        stop=True,
                    )
                    pt = ptpool.tile([128, QC], BF16)
                    nc.scalar.activation(pt, ps, EXP, scale=SCALE)
                    if d_off >= 0:
                        nc.vector.tensor_mul(pt, pt, masks[:, d_off, :])
                    nc.tensor.matmul(
                        pso,
                        lhsT=v_sb[b][:, kt, :],
                        rhs=pt,
                        start=(kt == 0),
                        stop=(kt == kt_max - 1),
                    )
                    if kt == 0:
                        nc.vector.tensor_copy(cs_a, pt)
                    elif kt == 1:
                        nc.gpsimd.tensor_copy(cs_b, pt)
                    elif kt % 2 == 0:
                        nc.vector.tensor_add(cs_a, cs_a, pt)
                    else:
                        nc.gpsimd.tensor_add(cs_b, cs_b, pt)
                # Ones-matmuls both sum over k and broadcast the sums to all
                # partitions (reusing a scores slot).
                sums_bc = ps_s.tile([128, QC], F32, tag="s", name="sums_bc")
                nc.tensor.matmul(sums_bc, lhsT=ones_mat, rhs=cs_a,
                                 start=True, stop=(kt_max < 2))
                if kt_max >= 2:
                    nc.tensor.matmul(sums_bc, lhsT=ones_mat, rhs=cs_b,
                                     start=False, stop=True)
                recip_bc = mpool.tile([128, QC], F32, tag="recip_bc")
                rscr = mpool.tile([128, QC], F32, tag="rscr")
                nc.vector.reciprocal_approx_accurate(recip_bc, sums_bc, rscr)
                nc.vector.tensor_mul(ag_stage[:, hh, :], pso, recip_bc)
            wr = nc.sync.dma_start(out=ag_ins[chunk][:], in_=ag_stage)
            pace["inst"] = wr.ins
            nc.gpsimd.collective_compute(
                "AllGather",
                mybir.AluOpType.bypass,
                replica_groups=[list(range(N_CORES))],
                ins=[ag_ins[chunk][:]],
                outs=[ag_outs[chunk][:]],
            )
            ags_fired.append(chunk)

        # --------------------- o_proj (one AG chunk) ------------------------
        # The AG-output load is emitted one attention chunk ahead of its
        # matmuls so the 2 MB DMA overlaps compute instead of bubbling the
        # in-order PE stream.
        ag_tiles = {}

        def emit_oproj_load(k):
            ag_sbt = apool.tile([128, N_HT, AGC], BF16)
            ld = nc.scalar.dma_start(
                out=ag_sbt.rearrange("p (r hl) t -> p r hl t", r=N_CORES),
                in_=ag_outs[k].rearrange("r p hl t -> p r hl t"),
            )
            if pace["inst"] is not None:
                # Pace AllGather-output consumption behind real attention
                # progress; the scheduler's collective latency estimate is
                # optimistic and would otherwise hoist this load (and its
                # dependent matmuls) too early in the PE stream.
                bass._add_dep_helper(
                    ld.ins, pace["inst"], sync=True,
                    reason="oproj paced behind attention",
                )
            ag_tiles[k] = ag_sbt

        def emit_oproj_mm(k):
            t0 = k * AGC
            ag_sbt = ag_tiles.pop(k)
            for jt in range(JC // 128):
                ps4 = ps_acc.tile([128, AGC], F32, tag="acc", name="ps4")
                for h in range(N_HT):
                    nc.tensor.matmul(
                        ps4,
                        lhsT=wo_sb[:, h, jt * 128:(jt + 1) * 128],
                        rhs=ag_sbt[:, h, :],
                        start=(h == 0),
                        stop=(h == N_HT - 1),
                    )
                res4 = respool.tile([128, AGC], BF16)
                nc.vector.tensor_copy(res4, ps4)
                nc.sync.dma_start(
                    out=out_t[jt * 128:(jt + 1) * 128, t0:t0 + AGC],
                    in_=res4,
                )
            oproj_done.append(k)

        # ------------------------- emission schedule ------------------------
        # P(ch): projection chunk; A(b,qc): attention q-chunk (needs proj
        # chunks 8b..8b+qc for K/V and 8b+qc for Q -> always >=2 chunks of
        # slack here).  o_proj k trails the AG stream by OPROJ_LAG.
        plan = [("P", 0), ("P", 1)]
        next_ch = 2
        for b in range(B):
            for qc in range(N_QC):
                plan.append(("A", b, qc))
                if next_ch < n_tc:
                    plan.append(("P", next_ch))
                    next_ch += 1
        # Reorder: keep P-chunks flowing a bit ahead of the A-groups they
        # feed.  plan above gives A(b,qc) after P(8b+qc+3).
        for item in plan:
            if item[0] == "P":
                proj_chunk(item[1])
            else:
                attn_group(item[1], item[2])
                n = len(ags_fired)
                if n > OPROJ_LAG:
                    emit_oproj_load(ags_fired[-1 - OPROJ_LAG])
                if n > OPROJ_LAG + 1:
                    emit_oproj_mm(ags_fired[-2 - OPROJ_LAG])
        for k in range(N_CH):
            if k not in ag_tiles and k not in oproj_done:
                emit_oproj_load(k)
                emit_oproj_mm(k)
            elif k in ag_tiles:
                emit_oproj_mm(k)


def _build_program():
    nc = bacc.Bacc("TRN2", target_bir_lowering=False, debug=False,
                   num_devices=N_CORES)
    xt = nc.declare_dram_parameter("xt", [128, T // TC, N_HT, TC], BF16,
                                   isOutput=False)
    wqkvt = nc.declare_dram_parameter("wqkvt", [128, HL + 2, N_HT, 128], BF16,
                                      isOutput=False)
    wot = nc.declare_dram_parameter("wot", [128, N_HT, JC], BF16,
                                    isOutput=False)
    ropes = nc.declare_dram_parameter("ropes", [128, T // TC, 2, TC], F32,
                                      isOutput=False)
    out_t = nc.declare_dram_parameter("out_t", [JC, T], BF16, isOutput=True)

    qkt = nc.dram_tensor("qkt", [HL * D, T], BF16).ap()
    ag_ins = [nc.dram_tensor(f"ag_in{k}", [128, HL, AGC], BF16).ap()
              for k in range(N_CH)]
    ag_outs = [nc.dram_tensor(f"ag_out{k}", [N_CORES, 128, HL, AGC], BF16,
                              addr_space="Shared").ap()
               for k in range(N_CH)]

    with tile.TileContext(nc) as tc_ctx:
        _emit(tc_ctx, xt, wqkvt, wot, ropes, out_t, qkt, ag_ins, ag_outs)
    nc.finalize()
    return nc


def _host_inputs(hidden_states, w_qkv, w_o):
    """Shard + transpose inputs for the 8 cores; returns in_maps."""
    X = np.asarray(hidden_states, dtype=np.float32).reshape(T, HID)
    # [p, ch, ht, tc] tiled layout so every DMA line is contiguous.
    xt = np.ascontiguousarray(
        X.reshape(T // TC, TC, N_HT, 128).transpose(3, 0, 2, 1)
    ).astype(ml_dtypes.bfloat16)

    # RoPE tables in [d, t] layout with rotate-half sign folded into sin and
    # the attention scale folded into the Q tables.
    inv_freq = 1.0 / (ROPE_BASE ** (np.arange(0, D, 2, dtype=np.float32) / D))
    pos = np.arange(S, dtype=np.float32)
    freqs = np.outer(pos, inv_freq)                      # (S, D/2)
    emb = np.concatenate([freqs, freqs], axis=-1)        # (S, D)
    cos = np.cos(emb).T.astype(np.float32)               # (D, S)
    sin = np.sin(emb).T.astype(np.float32)
    sgn = np.concatenate([-np.ones(D // 2), np.ones(D // 2)]).astype(np.float32)
    sins = sgn[:, None] * sin
    cos_t = np.tile(cos, (1, B))                         # (D, T)
    sins_t = np.tile(sins, (1, B))
    ropes = np.stack([cos_t, sins_t], axis=0)
    ropes = np.ascontiguousarray(
        ropes.reshape(2, 128, T // TC, TC).transpose(1, 2, 0, 3),
        dtype=np.float32,
    )

    w_qkv = np.asarray(w_qkv, dtype=np.float32)
    w_o = np.asarray(w_o, dtype=np.float32)
    q_sz = N_HEADS * D
    kv_sz = N_KV_HEADS * D
    in_maps = []
    for c in range(N_CORES):
        qr = w_qkv[c * HL * D:(c + 1) * HL * D]
        kr = w_qkv[q_sz + c * D:q_sz + (c + 1) * D]
        vr = w_qkv[q_sz + kv_sz + c * D:q_sz + kv_sz + (c + 1) * D]
        w_shard = np.concatenate([qr, kr, vr], axis=0)           # (768, HID)
        wqkvt_c = np.ascontiguousarray(
            w_shard.reshape(HL + 2, 128, N_HT, 128).transpose(3, 0, 2, 1)
        ).astype(ml_dtypes.bfloat16)
        wot_c = np.ascontiguousarray(
            w_o[c * JC:(c + 1) * JC, :].reshape(JC, N_HT, 128).transpose(2, 1, 0)
        ).astype(ml_dtypes.bfloat16)
        in_maps.append({
            "xt": xt, "wqkvt": wqkvt_c, "wot": wot_c, "ropes": ropes,
        })
    return in_maps


def _run(hidden_states, w_qkv, w_o, trace=False, tmpdir=None):
    in_maps = _host_inputs(hidden_states, w_qkv, w_o)
    nc = _build_program()
    res = run_bass_kernel_spmd(nc, in_maps, list(range(N_CORES)),
                               trace=trace, tmpdir=tmpdir)
    out_T = np.concatenate(
        [np.asarray(res.results[c]["out_t"]).astype(np.float32)
         for c in range(N_CORES)], axis=0
    )                                                     # (HID j, T)
    out = np.ascontiguousarray(out_T.T).reshape(B, S, HID).astype(np.float32)
    return out, res


def kernel(hidden_states, w_qkv, w_o):
    out, _ = _run(hidden_states, w_qkv, w_o, trace=False)
    return out
